# revision 27
# baseline (speedup 1.0000x reference)
"""Trainium2 Bass kernel for nn_Attention_54254026883778.

Single-head attention with an additive post-softmax intensity term:
    q/k/v = X @ W{q,k,v}.T + b;  scores = q k^T / sqrt(D)
    attn  = softmax(scores) + intensity;  out = (attn @ v) @ Wo.T + bo

Sharding: 8 cores = 4 batches x 2 sequence halves. Each core computes
Q for its own 1024 query rows and K for the WHOLE batch locally (K is
cheap in fp8, so duplicating it across the pair beats a collective).
V' is computed for the own half and exchanged through two staged 2-rank
AllGathers (the only collectives).

Math restructuring (host side, exact):
    Wvo = Wo @ Wv  =>  out = attn @ (X @ Wvo.T) + rowsum(attn) (x) (Wo@bv) + bo
which deletes the output projection GEMM. rowsum(attn) = 1 + rowsum(I)
is shipped from the host.

Precision: all large GEMMs run in fp8 e4m3 with MatmulPerfMode.DoubleRow
(256-row contraction per instruction, 2x bf16 throughput):
  - Q/K projections + scores: weights pre-scaled by 32 on the host so
    their uniform(+-1/32) entries stay in the e4m3 normal range; the
    32*32 factor is folded into the softmax exp scale. The softmax
    contribution to the output is tiny (intensity dominates attn).
  - PV runs fp8 on DOUBLE-CENTERED operands: attn_c = attn - 0.5 (the
    host ships intensity^T - 0.5) and V_c = V' - m, where m is the
    column mean of V' (host-exact: m = colsum(X) @ Wvo.T / S, quantized
    to the same bf16 the device subtracts). Centering routes the large
    attn/V' means through the exact f32 bias tensor instead of the fp8
    operands, which is what keeps the quantization noise acceptable:
       BIAS[do,s] = (Wo@bv + m)[do] * rowsums[s] + bo[do]
    (the m (x) rowsums term reconstructs both centering corrections).
  - V'-proj itself stays bf16 (X, Wvo bf16): V'-path input errors are
    amplified by ~sqrt(S)*|attn| in attn @ V', so fp8 inputs there
    would blow the error budget; fp8 only on the PV operand side.

Device dataflow (all t in global order; per-core tensors from host):
    K^T  [dout | t]   = WK8-chunk.T @ XF8      (fp8 DR, full S)
    V'   [t-own | dv] = XH16-chunk.T @ WVO16   (bf16, own half) - m -> fp8
        -> two staged AllGathers (t-chunks 0..3, then 4..7) -> V full
    Q^T  [dout | s]   = WQ8-chunk.T @ XQ8      (fp8 DR, own rows)
    scores [s | t]    = QT-chunk.T @ KT        (fp8 DR) -> exp on ACT
        (scale 1/32768, no max-subtract: |scores|<~3) with fused
        row-accumulate -> 1/den on DVE -> diag(recip) = ident * recip ->
        attn^T tile = E-slice.T @ diag(recip)  (one PE matmul both
        transposes and normalizes) -> DVE drain adds (I^T - 0.5) -> fp8
        The transpose block for si runs after the scores block of si+1
        so the PE never waits on the exp->recip->diag chain.
    out^T [do | s]    = V-chunk.T @ attn_c^T   (fp8 DR) -> DVE adds the
        host BIAS tile while draining PSUM -> DRAM (f32), host transposes.
"""

import numpy as np
import ml_dtypes

P = 128
D = 1024
S = 2048          # keys per batch (full sequence)
SH = 1024         # query rows owned by each core
DC = D // P       # 8  contraction chunks over model dim
DC2 = DC // 2     # 4  DoubleRow chunk-pairs
TC = S // P       # 16 t (key) chunks
TC2 = TC // 2     # 8  DoubleRow t-chunk pairs
NT = 512          # matmul moving free dim / psum bank
SJ = SH // NT     # 2  s-tiles of own rows
TJ = S // NT      # 4  t-tiles
WS = 32.0         # host pre-scale on Wq/Wk (keeps fp8 in normal range)
SCALE = 1.0 / (32.0 * WS * WS)  # 1/sqrt(D) / WS^2

_CACHE = {}


def _build_module():
    import concourse.bass as bass
    import concourse.tile as tile
    import concourse.mybir as mybir
    from concourse import bacc
    from concourse.masks import make_identity

    f32 = mybir.dt.float32
    bf16 = mybir.dt.bfloat16
    fp8 = mybir.dt.float8e4
    DR = mybir.MatmulPerfMode.DoubleRow
    Exp = mybir.ActivationFunctionType.Exp
    add = mybir.AluOpType.add
    sub = mybir.AluOpType.subtract

    nc = bacc.Bacc("TRN2", target_bir_lowering=False, debug=False,
                   num_devices=8)

    XQ_d = nc.dram_tensor("XQ8", [D, SH], fp8, kind="ExternalInput")
    XH_d = nc.dram_tensor("XH16", [D, SH], bf16, kind="ExternalInput")
    WQ_d = nc.dram_tensor("WQ8", [D, D], fp8, kind="ExternalInput")
    WK_d = nc.dram_tensor("WK8", [D, D], fp8, kind="ExternalInput")
    WVO_d = nc.dram_tensor("WVO16", [D, D], bf16, kind="ExternalInput")
    M_d = nc.dram_tensor("M16", [P, D], bf16, kind="ExternalInput")
    BCOL_d = nc.dram_tensor("BCOL", [P, 2 * DC], f32, kind="ExternalInput")
    BIAS_d = nc.dram_tensor("BIAS", [D, SH], f32, kind="ExternalInput")
    IT_d = nc.dram_tensor("IT16", [SH, S], bf16, kind="ExternalInput")
    OUT_d = nc.dram_tensor("OUTT", [D, SH], f32, kind="ExternalOutput")

    xq_v = XQ_d[:].rearrange("(c p) s -> p c s", p=P)
    xh_v = XH_d[:].rearrange("(c p) s -> p c s", p=P)
    wq_v = WQ_d[:].rearrange("(c p) o -> p c o", p=P)
    wk_v = WK_d[:].rearrange("(c p) o -> p c o", p=P)
    wvo_v = WVO_d[:].rearrange("(c p) o -> p c o", p=P)
    it_v = IT_d[:].rearrange("(si p) f -> si p f", p=P)   # per-si blocks
    bias_v = BIAS_d[:].rearrange("(c p) s -> p c s", p=P)
    out_v = OUT_d[:].rearrange("(c p) s -> p c s", p=P)

    GROUPS = [[0, 1], [2, 3], [4, 5], [6, 7]]

    with tile.TileContext(nc) as tc:
        with (
            tc.tile_pool(name="persist", bufs=1) as persist,
            tc.tile_pool(name="mm_ps", bufs=5, space="PSUM") as mm_ps,
            tc.tile_pool(name="tr_ps", bufs=3, space="PSUM") as tr_ps,
            tc.tile_pool(name="dram", bufs=1, space="DRAM") as dram_pool,
            tc.tile_pool(name="e_pool", bufs=2) as e_pool,
            tc.tile_pool(name="it_pool", bufs=3) as it_pool,
            tc.tile_pool(name="stat_pool", bufs=2) as stat_pool,
            tc.tile_pool(name="bias_pool", bufs=8) as bias_pool,
            tc.tile_pool(name="fin_pool", bufs=3) as fin_pool,
        ):
            # ---- persistent tiles -------------------------------------
            WK_sb = persist.tile([P, DC, D], fp8)
            WQ_sb = persist.tile([P, DC, D], fp8)
            KL_sb = persist.tile([P, DC, SH], fp8)     # own K^T [dout | t]
            XQ_sb = persist.tile([P, DC, SH], fp8)
            WVO_sb = persist.tile([P, DC, D], bf16)
            XH_sb = persist.tile([P, DC, SH], bf16)
            # K^T [dout | rank-half, gather-half, dc-in-half, t]; the
            # extra split keeps each gather-output copy contiguous per
            # partition (4KB lines).
            KT_sb = persist.tile([P, 2, 2, DC // 2, SH], fp8)
            QT_sb = persist.tile([P, DC, SH], fp8)     # Q^T [dout | s]
            VL_sb = persist.tile([P, DC, SH], fp8)     # own V_c [t | dv]
            V_sb = persist.tile([P, TC, D], fp8)       # full V_c [t | dv]
            AT_sb = persist.tile([P, TC, SH], fp8)     # attn_c^T [t | s]
            M_sb = persist.tile([P, D], bf16)          # colmean(V') rows
            ident = persist.tile([P, P], bf16)
            BCOL_sb = persist.tile([P, 2 * DC], f32)

            # K operands first (the K exchange is the longest dependency
            # chain), then V', then Q. The first matmul's payload is
            # split extra-fine (WK pair0 col-block + XQ pair0 halves) so
            # the PE starts as soon as ~160KB has landed. Chunk-pair
            # loads keep per-partition lines at 2KB+ for DMA efficiency.
            nc.sync.dma_start(WK_sb[:, 0:2, 0:P], wk_v[:, 0:2, 0:P])
            for sj in range(SJ):
                nc.sync.dma_start(XQ_sb[:, 0:2, sj * NT:(sj + 1) * NT],
                                  xq_v[:, 0:2, sj * NT:(sj + 1) * NT])
            nc.sync.dma_start(WK_sb[:, 0:2, P:D], wk_v[:, 0:2, P:D])
            for dc2 in range(1, DC2):
                nc.sync.dma_start(WK_sb[:, 2 * dc2:2 * dc2 + 2, :],
                                  wk_v[:, 2 * dc2:2 * dc2 + 2, :])
                nc.sync.dma_start(XQ_sb[:, 2 * dc2:2 * dc2 + 2, :],
                                  xq_v[:, 2 * dc2:2 * dc2 + 2, :])
            make_identity(nc, ident)
            nc.sync.dma_start(BCOL_sb[:], BCOL_d[:])
            nc.sync.dma_start(M_sb[:], M_d[:])
            for dc2 in range(DC2):
                nc.sync.dma_start(WQ_sb[:, 2 * dc2:2 * dc2 + 2, :],
                                  wq_v[:, 2 * dc2:2 * dc2 + 2, :])
            for dc in range(DC):
                nc.sync.dma_start(WVO_sb[:, dc, :], wvo_v[:, dc, :])
                nc.sync.dma_start(XH_sb[:, dc, :], xh_v[:, dc, :])

            # ---- K^T (fp8 DoubleRow) own rows, then pair AllGather ----
            # The exchange goes out in two 0.5MB halves (dout-chunks 0-3
            # and 4-7) so the first gather can start as soon as the
            # runtime's init barrier clears the CC stream.
            HK = DC // 2
            for c in range(DC):
                psl = [mm_ps.tile([P, NT], f32, tag="mm", name="ps")
                       for _ in range(SJ)]
                for dc2 in range(DC2):
                    for sj in range(SJ):
                        nc.tensor.matmul(
                            psl[sj][:],
                            WK_sb[:, 2 * dc2:2 * dc2 + 2, c * P:(c + 1) * P],
                            XQ_sb[:, 2 * dc2:2 * dc2 + 2, sj * NT:(sj + 1) * NT],
                            start=(dc2 == 0), stop=(dc2 == DC2 - 1),
                            perf_mode=DR,
                        )
                for sj in range(SJ):
                    nc.vector.tensor_scalar_add(
                        KL_sb[:, c, sj * NT:(sj + 1) * NT], psl[sj][:],
                        BCOL_sb[:, DC + c:DC + c + 1])
                if c % HK == HK - 1:
                    kh = c // HK
                    k_in = dram_pool.tile([P, HK, SH], fp8, name=f"k_in{kh}")
                    k_out = dram_pool.tile([2, P, HK, SH], fp8,
                                           name=f"k_out{kh}")
                    nc.sync.dma_start(k_in[:], KL_sb[:, kh * HK:(kh + 1) * HK])
                    nc.gpsimd.collective_compute(
                        "AllGather", mybir.AluOpType.bypass,
                        replica_groups=GROUPS,
                        ins=[k_in.opt()], outs=[k_out.opt()])
                    # rank order == global t order: half 0 even, half 1 odd
                    nc.sync.dma_start(KT_sb[:, 0, kh], k_out[0])
                    nc.sync.dma_start(KT_sb[:, 1, kh], k_out[1])

            # ---- Q^T (fp8 DoubleRow), own rows ------------------------
            for c in range(DC):
                psl = [mm_ps.tile([P, NT], f32, tag="mm", name="ps")
                       for _ in range(SJ)]
                for dc2 in range(DC2):
                    for sj in range(SJ):
                        nc.tensor.matmul(
                            psl[sj][:],
                            WQ_sb[:, 2 * dc2:2 * dc2 + 2, c * P:(c + 1) * P],
                            XQ_sb[:, 2 * dc2:2 * dc2 + 2, sj * NT:(sj + 1) * NT],
                            start=(dc2 == 0), stop=(dc2 == DC2 - 1),
                            perf_mode=DR,
                        )
                for sj in range(SJ):
                    nc.vector.tensor_scalar_add(
                        QT_sb[:, c, sj * NT:(sj + 1) * NT], psl[sj][:],
                        BCOL_sb[:, c:c + 1])

            # Prefetch the first IT tiles now so their DMAs sit ahead
            # of the V-exchange staging/copies in the sync queues.
            IT_tiles = {}
            for si in range(3):
                IT_tiles[si] = it_pool.tile([P, S], bf16, tag="it",
                                            name=f"it{si}")
                nc.sync.dma_start(IT_tiles[si][:], it_v[si])

            # ---- V' (bf16) own half, centered -> fp8; staged exchange -
            for half in range(2):
                for t in range(half * DC // 2, (half + 1) * DC // 2):
                    psl = [mm_ps.tile([P, NT], f32, tag="mm", name="ps")
                           for _ in range(D // NT)]
                    for dc in range(DC):
                        for j in range(D // NT):
                            nc.tensor.matmul(
                                psl[j][:],
                                XH_sb[:, dc, t * P:(t + 1) * P],
                                WVO_sb[:, dc, j * NT:(j + 1) * NT],
                                start=(dc == 0), stop=(dc == DC - 1),
                            )
                    for j in range(D // NT):
                        nc.vector.tensor_tensor(
                            VL_sb[:, t, j * NT:(j + 1) * NT],
                            psl[j][:], M_sb[:, j * NT:(j + 1) * NT], sub)
                HC = DC // 2
                v_in = dram_pool.tile([P, HC, SH], fp8, name=f"v_in{half}")
                v_out = dram_pool.tile([2, P, HC, SH], fp8,
                                       name=f"v_out{half}")
                # Stage via HWDGE (SWDGE staging adds ~20us of trigger
                # latency); only the collective itself runs on gpsimd.
                nc.sync.dma_start(v_in[:], VL_sb[:, half * HC:(half + 1) * HC])
                nc.gpsimd.collective_compute(
                    "AllGather", mybir.AluOpType.bypass,
                    replica_groups=GROUPS,
                    ins=[v_in.opt()], outs=[v_out.opt()])
                nc.sync.dma_start(V_sb[:, half * HC:(half + 1) * HC], v_out[0])
                nc.sync.dma_start(V_sb[:, DC + half * HC:DC + (half + 1) * HC],
                                  v_out[1])

            # ---- scores -> softmax -> +(I^T-0.5) -> attn_c^T (fp8) ----
            # Transposes for si are emitted after the scores block of
            # si+1, so diag(si) is ready by the time the PE gets there.
            # BIAS tiles for the PV phase prefetch during this window.
            B_tiles = []
            for dvi in range(DC):
                B_sb = bias_pool.tile([P, SH], f32, tag="bias")
                nc.scalar.dma_start(B_sb[:], bias_v[:, dvi, :])
                B_tiles.append(B_sb)
            if True:
                pend = None  # (si, E_sb, IT_sb, diag)

                def emit_transposes(si, E_sb, IT_sb, diag):
                    for t in range(TC):
                        pt = tr_ps.tile([P, P], f32, tag="tr")
                        nc.tensor.matmul(
                            pt[:], E_sb[:, t * P:(t + 1) * P], diag[:],
                            start=True, stop=True)
                        nc.vector.tensor_tensor(
                            AT_sb[:, t, si * P:(si + 1) * P],
                            pt[:], IT_sb[:, t * P:(t + 1) * P], add)

                for si in range(DC):
                    if si in IT_tiles:
                        IT_sb = IT_tiles[si]
                    else:
                        IT_sb = it_pool.tile([P, S], bf16, tag="it")
                        nc.sync.dma_start(IT_sb[:], it_v[si])
                    E_sb = e_pool.tile([P, S], bf16, tag="e")
                    ACC = stat_pool.tile([P, TJ], f32, tag="acc")
                    psl = [mm_ps.tile([P, NT], f32, tag="mm", name="ps")
                           for _ in range(TJ)]
                    for dc2 in range(DC2):
                        for tj in range(TJ):
                            nc.tensor.matmul(
                                psl[tj][:],
                                QT_sb[:, 2 * dc2:2 * dc2 + 2, si * P:(si + 1) * P],
                                KT_sb[:, tj // 2, dc2 // 2,
                                      2 * (dc2 % 2):2 * (dc2 % 2) + 2,
                                      (tj % 2) * NT:(tj % 2 + 1) * NT],
                                start=(dc2 == 0), stop=(dc2 == DC2 - 1),
                                perf_mode=DR,
                            )
                    for tj in range(TJ):
                        nc.scalar.activation(
                            E_sb[:, tj * NT:(tj + 1) * NT], psl[tj][:], Exp,
                            scale=SCALE, accum_out=ACC[:, tj:tj + 1],
                        )
                    den = stat_pool.tile([P, 1], f32, tag="den")
                    recip = stat_pool.tile([P, 1], f32, tag="recip")
                    diag = stat_pool.tile([P, P], bf16, tag="diag")
                    nc.vector.reduce_sum(
                        den[:], ACC[:], axis=mybir.AxisListType.X)
                    nc.vector.reciprocal(recip[:], den[:])
                    nc.vector.tensor_scalar_mul(diag[:], ident[:], recip[:])
                    if pend is not None:
                        emit_transposes(*pend)
                    pend = (si, E_sb, IT_sb, diag)
                emit_transposes(*pend)

            # ---- PV (fp8 DoubleRow): out^T = V_c.T @ attn_c^T + BIAS --
            for dvi in range(DC):
                B_sb = B_tiles[dvi]
                psl = [mm_ps.tile([P, NT], f32, tag="mm", name="ps")
                       for _ in range(SJ)]
                for tc2 in range(TC2):
                    for sj in range(SJ):
                        nc.tensor.matmul(
                            psl[sj][:],
                            V_sb[:, 2 * tc2:2 * tc2 + 2, dvi * P:(dvi + 1) * P],
                            AT_sb[:, 2 * tc2:2 * tc2 + 2, sj * NT:(sj + 1) * NT],
                            start=(tc2 == 0), stop=(tc2 == TC2 - 1),
                            perf_mode=DR,
                        )
                for sj in range(SJ):
                    F_sb = fin_pool.tile([P, NT], f32, tag="fin")
                    nc.vector.tensor_tensor(
                        F_sb[:], psl[sj][:],
                        B_sb[:, sj * NT:(sj + 1) * NT], add)
                    nc.sync.dma_start(
                        out_v[:, dvi, sj * NT:(sj + 1) * NT], F_sb[:])

    nc.compile()
    return nc


def _get_module():
    if "nc" not in _CACHE:
        _CACHE["nc"] = _build_module()
    return _CACHE["nc"]


def _make_in_maps(inputs):
    X = np.asarray(inputs["X"], dtype=np.float32)
    intensity = np.asarray(inputs["intensity"], dtype=np.float32)
    bf = ml_dtypes.bfloat16
    f8 = ml_dtypes.float8_e4m3
    Wq = np.asarray(inputs["Wq"], np.float32)
    Wk = np.asarray(inputs["Wk"], np.float32)
    Wv = np.asarray(inputs["Wv"], np.float32)
    Wo = np.asarray(inputs["Wo"], np.float32)
    Wvo = (Wo.astype(np.float64) @ Wv.astype(np.float64))  # fused V/O proj
    WQ8 = np.ascontiguousarray((WS * Wq).T).astype(f8)
    WK8 = np.ascontiguousarray((WS * Wk).T).astype(f8)
    WVO16 = np.ascontiguousarray(Wvo.T.astype(np.float32)).astype(bf)
    bq, bk, bv, bo = (np.asarray(inputs[k], np.float32).reshape(D)
                      for k in ("bq", "bk", "bv", "bo"))
    bvo = (Wo.astype(np.float64) @ bv.astype(np.float64))
    BCOL = np.concatenate(
        [(WS * b).reshape(DC, P).T for b in (bq, bk)], axis=1
    ).astype(np.float32)  # [128, 16]

    in_maps = []
    for c in range(8):
        b, h = c // 2, c % 2
        XT = X[b].T                                   # [D, S]
        XQ8 = np.ascontiguousarray(XT[:, h * SH:(h + 1) * SH]).astype(f8)
        XH16 = np.ascontiguousarray(XT[:, h * SH:(h + 1) * SH]).astype(bf)
        # column mean of V' = colsum(X) @ Wvo.T / S, quantized to the
        # same bf16 the device subtracts so bias and centering agree.
        m = (X[b].sum(axis=0, dtype=np.float64) @ Wvo.T) / S
        m16 = m.astype(np.float32).astype(bf)
        M16 = np.broadcast_to(m16, (P, D)).copy()
        Islc = intensity[b, h * SH:(h + 1) * SH, :]
        # [t, s] -> [si*128+tp, tc*128+sp] so each per-si load is one
        # contiguous row-block (128 descriptors instead of 2048)
        IT16 = np.ascontiguousarray(
            (Islc.T - 0.5).reshape(TC, P, DC, P)
            .transpose(2, 1, 0, 3).reshape(SH, S)
        ).astype(bf)
        rows = 1.0 + Islc.sum(axis=1, dtype=np.float64)
        BIAS = ((bvo + m16.astype(np.float64))[:, None] * rows[None, :]
                + bo.astype(np.float64)[:, None]).astype(np.float32)
        in_maps.append({
            "XQ8": XQ8, "XH16": XH16,
            "WQ8": WQ8, "WK8": WK8, "WVO16": WVO16, "M16": M16,
            "BCOL": BCOL, "BIAS": BIAS, "IT16": IT16,
        })
    return in_maps


def _gather(results):
    out = np.empty((4, S, D), dtype=np.float32)
    for c in range(8):
        b, h = c // 2, c % 2
        out[b, h * SH:(h + 1) * SH, :] = results[c]["OUTT"].T
    return out


def kernel(**inputs):
    from concourse import bass_utils

    in_maps = _make_in_maps(inputs)
    nc = _get_module()
    res = bass_utils.run_bass_kernel_spmd(nc, in_maps, core_ids=list(range(8)))
    return _gather(res.results)


# revision 28
# speedup vs baseline: 1.0015x; 1.0015x over previous
"""Trainium2 Bass kernel for nn_Attention_54254026883778.

Single-head attention with an additive post-softmax intensity term:
    q/k/v = X @ W{q,k,v}.T + b;  scores = q k^T / sqrt(D)
    attn  = softmax(scores) + intensity;  out = (attn @ v) @ Wo.T + bo

Sharding: 8 cores = 4 batches x 2 sequence halves. Each core computes
Q for its own 1024 query rows and K for the WHOLE batch locally (K is
cheap in fp8, so duplicating it across the pair beats a collective).
V' is computed for the own half and exchanged through two staged 2-rank
AllGathers (the only collectives).

Math restructuring (host side, exact):
    Wvo = Wo @ Wv  =>  out = attn @ (X @ Wvo.T) + rowsum(attn) (x) (Wo@bv) + bo
which deletes the output projection GEMM. rowsum(attn) = 1 + rowsum(I)
is shipped from the host.

Precision: all large GEMMs run in fp8 e4m3 with MatmulPerfMode.DoubleRow
(256-row contraction per instruction, 2x bf16 throughput):
  - Q/K projections + scores: weights pre-scaled by 32 on the host so
    their uniform(+-1/32) entries stay in the e4m3 normal range; the
    32*32 factor is folded into the softmax exp scale. The softmax
    contribution to the output is tiny (intensity dominates attn).
  - PV runs fp8 on DOUBLE-CENTERED operands: attn_c = attn - 0.5 (the
    host ships intensity^T - 0.5) and V_c = V' - m, where m is the
    column mean of V' (host-exact: m = colsum(X) @ Wvo.T / S, quantized
    to the same bf16 the device subtracts). Centering routes the large
    attn/V' means through the exact f32 bias tensor instead of the fp8
    operands, which is what keeps the quantization noise acceptable:
       BIAS[do,s] = (Wo@bv + m)[do] * rowsums[s] + bo[do]
    (the m (x) rowsums term reconstructs both centering corrections).
  - V'-proj itself stays bf16 (X, Wvo bf16): V'-path input errors are
    amplified by ~sqrt(S)*|attn| in attn @ V', so fp8 inputs there
    would blow the error budget; fp8 only on the PV operand side.

Device dataflow (all t in global order; per-core tensors from host):
    K^T  [dout | t]   = WK8-chunk.T @ XF8      (fp8 DR, full S)
    V'   [t-own | dv] = XH16-chunk.T @ WVO16   (bf16, own half) - m -> fp8
        -> two staged AllGathers (t-chunks 0..3, then 4..7) -> V full
    Q^T  [dout | s]   = WQ8-chunk.T @ XQ8      (fp8 DR, own rows)
    scores [s | t]    = QT-chunk.T @ KT        (fp8 DR) -> exp on ACT
        (scale 1/32768, no max-subtract: |scores|<~3) with fused
        row-accumulate -> 1/den on DVE -> diag(recip) = ident * recip ->
        attn^T tile = E-slice.T @ diag(recip)  (one PE matmul both
        transposes and normalizes) -> DVE drain adds (I^T - 0.5) -> fp8
        The transpose block for si runs after the scores block of si+1
        so the PE never waits on the exp->recip->diag chain.
    out^T [do | s]    = V-chunk.T @ attn_c^T   (fp8 DR) -> DVE adds the
        host BIAS tile while draining PSUM -> DRAM (f32), host transposes.
"""

import numpy as np
import ml_dtypes

P = 128
D = 1024
S = 2048          # keys per batch (full sequence)
SH = 1024         # query rows owned by each core
DC = D // P       # 8  contraction chunks over model dim
DC2 = DC // 2     # 4  DoubleRow chunk-pairs
TC = S // P       # 16 t (key) chunks
TC2 = TC // 2     # 8  DoubleRow t-chunk pairs
NT = 512          # matmul moving free dim / psum bank
SJ = SH // NT     # 2  s-tiles of own rows
TJ = S // NT      # 4  t-tiles
WS = 32.0         # host pre-scale on Wq/Wk (keeps fp8 in normal range)
SCALE = 1.0 / (32.0 * WS * WS)  # 1/sqrt(D) / WS^2

_CACHE = {}


def _build_module():
    import concourse.bass as bass
    import concourse.tile as tile
    import concourse.mybir as mybir
    from concourse import bacc
    from concourse.masks import make_identity

    f32 = mybir.dt.float32
    bf16 = mybir.dt.bfloat16
    fp8 = mybir.dt.float8e4
    DR = mybir.MatmulPerfMode.DoubleRow
    Exp = mybir.ActivationFunctionType.Exp
    add = mybir.AluOpType.add
    sub = mybir.AluOpType.subtract

    nc = bacc.Bacc("TRN2", target_bir_lowering=False, debug=False,
                   num_devices=8)

    XQ_d = nc.dram_tensor("XQ8", [D, SH], fp8, kind="ExternalInput")
    XH_d = nc.dram_tensor("XH16", [D, SH], bf16, kind="ExternalInput")
    WQ_d = nc.dram_tensor("WQ8", [D, D], fp8, kind="ExternalInput")
    WK_d = nc.dram_tensor("WK8", [D, D], fp8, kind="ExternalInput")
    WVO_d = nc.dram_tensor("WVO16", [D, D], bf16, kind="ExternalInput")
    M_d = nc.dram_tensor("M16", [P, D], bf16, kind="ExternalInput")
    BCOL_d = nc.dram_tensor("BCOL", [P, 2 * DC], f32, kind="ExternalInput")
    BIAS_d = nc.dram_tensor("BIAS", [D, SH], f32, kind="ExternalInput")
    IT_d = nc.dram_tensor("IT16", [SH, S], bf16, kind="ExternalInput")
    OUT_d = nc.dram_tensor("OUTT", [D, SH], f32, kind="ExternalOutput")

    xq_v = XQ_d[:].rearrange("(c p) s -> p c s", p=P)
    xh_v = XH_d[:].rearrange("(c p) s -> p c s", p=P)
    wq_v = WQ_d[:].rearrange("(c p) o -> p c o", p=P)
    wk_v = WK_d[:].rearrange("(c p) o -> p c o", p=P)
    wvo_v = WVO_d[:].rearrange("(c p) o -> p c o", p=P)
    it_v = IT_d[:].rearrange("(si p) f -> si p f", p=P)   # per-si blocks
    bias_v = BIAS_d[:].rearrange("(c p) s -> p c s", p=P)
    out_v = OUT_d[:].rearrange("(c p) s -> p c s", p=P)

    GROUPS = [[0, 1], [2, 3], [4, 5], [6, 7]]

    with tile.TileContext(nc) as tc:
        with (
            tc.tile_pool(name="persist", bufs=1) as persist,
            tc.tile_pool(name="mm_ps", bufs=5, space="PSUM") as mm_ps,
            tc.tile_pool(name="tr_ps", bufs=3, space="PSUM") as tr_ps,
            tc.tile_pool(name="dram", bufs=1, space="DRAM") as dram_pool,
            tc.tile_pool(name="e_pool", bufs=2) as e_pool,
            tc.tile_pool(name="it_pool", bufs=3) as it_pool,
            tc.tile_pool(name="stat_pool", bufs=2) as stat_pool,
            tc.tile_pool(name="bias_pool", bufs=8) as bias_pool,
            tc.tile_pool(name="fin_pool", bufs=3) as fin_pool,
        ):
            # ---- persistent tiles -------------------------------------
            WK_sb = persist.tile([P, DC, D], fp8)
            WQ_sb = persist.tile([P, DC, D], fp8)
            KL_sb = persist.tile([P, DC, SH], fp8)     # own K^T [dout | t]
            XQ_sb = persist.tile([P, DC, SH], fp8)
            WVO_sb = persist.tile([P, DC, D], bf16)
            XH_sb = persist.tile([P, DC, SH], bf16)
            # K^T [dout | rank-half, gather-half, dc-in-half, t]; the
            # extra split keeps each gather-output copy contiguous per
            # partition (4KB lines).
            KT_sb = persist.tile([P, 2, 2, DC // 2, SH], fp8)
            QT_sb = persist.tile([P, DC, SH], fp8)     # Q^T [dout | s]
            VL_sb = persist.tile([P, DC, SH], fp8)     # own V_c [t | dv]
            V_sb = persist.tile([P, TC, D], fp8)       # full V_c [t | dv]
            AT_sb = persist.tile([P, TC, SH], fp8)     # attn_c^T [t | s]
            M_sb = persist.tile([P, D], bf16)          # colmean(V') rows
            ident = persist.tile([P, P], bf16)
            BCOL_sb = persist.tile([P, 2 * DC], f32)

            # K operands first (the K exchange is the longest dependency
            # chain), then V', then Q. The first matmul's payload is
            # split extra-fine (WK pair0 col-block + XQ pair0 halves) so
            # the PE starts as soon as ~160KB has landed. Chunk-pair
            # loads keep per-partition lines at 2KB+ for DMA efficiency.
            nc.sync.dma_start(WK_sb[:, 0:2, 0:P], wk_v[:, 0:2, 0:P])
            for sj in range(SJ):
                nc.sync.dma_start(XQ_sb[:, 0:2, sj * NT:(sj + 1) * NT],
                                  xq_v[:, 0:2, sj * NT:(sj + 1) * NT])
            nc.sync.dma_start(WK_sb[:, 0:2, P:D], wk_v[:, 0:2, P:D])
            for dc2 in range(1, DC2):
                nc.sync.dma_start(WK_sb[:, 2 * dc2:2 * dc2 + 2, :],
                                  wk_v[:, 2 * dc2:2 * dc2 + 2, :])
                nc.sync.dma_start(XQ_sb[:, 2 * dc2:2 * dc2 + 2, :],
                                  xq_v[:, 2 * dc2:2 * dc2 + 2, :])
            make_identity(nc, ident)
            nc.sync.dma_start(BCOL_sb[:], BCOL_d[:])
            nc.sync.dma_start(M_sb[:], M_d[:])
            for dc2 in range(DC2):
                nc.sync.dma_start(WQ_sb[:, 2 * dc2:2 * dc2 + 2, :],
                                  wq_v[:, 2 * dc2:2 * dc2 + 2, :])
            for dc in range(DC):
                nc.sync.dma_start(WVO_sb[:, dc, :], wvo_v[:, dc, :])
                nc.sync.dma_start(XH_sb[:, dc, :], xh_v[:, dc, :])

            # ---- K^T (fp8 DoubleRow) own rows, then pair AllGather ----
            # The exchange goes out in two 0.5MB halves (dout-chunks 0-3
            # and 4-7) so the first gather can start as soon as the
            # runtime's init barrier clears the CC stream.
            HK = DC // 2
            for c in range(DC):
                psl = [mm_ps.tile([P, NT], f32, tag="mm", name="ps")
                       for _ in range(SJ)]
                for dc2 in range(DC2):
                    for sj in range(SJ):
                        nc.tensor.matmul(
                            psl[sj][:],
                            WK_sb[:, 2 * dc2:2 * dc2 + 2, c * P:(c + 1) * P],
                            XQ_sb[:, 2 * dc2:2 * dc2 + 2, sj * NT:(sj + 1) * NT],
                            start=(dc2 == 0), stop=(dc2 == DC2 - 1),
                            perf_mode=DR,
                        )
                for sj in range(SJ):
                    nc.vector.tensor_scalar_add(
                        KL_sb[:, c, sj * NT:(sj + 1) * NT], psl[sj][:],
                        BCOL_sb[:, DC + c:DC + c + 1])
                if c % HK == HK - 1:
                    kh = c // HK
                    k_in = dram_pool.tile([P, HK, SH], fp8, name=f"k_in{kh}")
                    k_out = dram_pool.tile([2, P, HK, SH], fp8,
                                           name=f"k_out{kh}")
                    nc.sync.dma_start(k_in[:], KL_sb[:, kh * HK:(kh + 1) * HK])
                    nc.gpsimd.collective_compute(
                        "AllGather", mybir.AluOpType.bypass,
                        replica_groups=GROUPS,
                        ins=[k_in.opt()], outs=[k_out.opt()])
                    # rank order == global t order: half 0 even, half 1 odd
                    nc.sync.dma_start(KT_sb[:, 0, kh], k_out[0])
                    nc.sync.dma_start(KT_sb[:, 1, kh], k_out[1])

            # ---- Q^T (fp8 DoubleRow), own rows ------------------------
            for c in range(DC):
                psl = [mm_ps.tile([P, NT], f32, tag="mm", name="ps")
                       for _ in range(SJ)]
                for dc2 in range(DC2):
                    for sj in range(SJ):
                        nc.tensor.matmul(
                            psl[sj][:],
                            WQ_sb[:, 2 * dc2:2 * dc2 + 2, c * P:(c + 1) * P],
                            XQ_sb[:, 2 * dc2:2 * dc2 + 2, sj * NT:(sj + 1) * NT],
                            start=(dc2 == 0), stop=(dc2 == DC2 - 1),
                            perf_mode=DR,
                        )
                for sj in range(SJ):
                    nc.vector.tensor_scalar_add(
                        QT_sb[:, c, sj * NT:(sj + 1) * NT], psl[sj][:],
                        BCOL_sb[:, c:c + 1])

            # Prefetch the first IT tiles now so their DMAs sit ahead
            # of the V-exchange staging/copies in the sync queues.
            IT_tiles = {}
            for si in range(3):
                IT_tiles[si] = it_pool.tile([P, S], bf16, tag="it",
                                            name=f"it{si}")
                nc.sync.dma_start(IT_tiles[si][:], it_v[si])

            # ---- V' (bf16) own half, centered -> fp8; staged exchange -
            for half in range(2):
                for t in range(half * DC // 2, (half + 1) * DC // 2):
                    psl = [mm_ps.tile([P, NT], f32, tag="mm", name="ps")
                           for _ in range(D // NT)]
                    for dc in range(DC):
                        for j in range(D // NT):
                            nc.tensor.matmul(
                                psl[j][:],
                                XH_sb[:, dc, t * P:(t + 1) * P],
                                WVO_sb[:, dc, j * NT:(j + 1) * NT],
                                start=(dc == 0), stop=(dc == DC - 1),
                            )
                    for j in range(D // NT):
                        nc.vector.tensor_tensor(
                            VL_sb[:, t, j * NT:(j + 1) * NT],
                            psl[j][:], M_sb[:, j * NT:(j + 1) * NT], sub)
                HC = DC // 2
                v_in = dram_pool.tile([P, HC, SH], fp8, name=f"v_in{half}")
                v_out = dram_pool.tile([2, P, HC, SH], fp8,
                                       name=f"v_out{half}")
                # Stage via HWDGE (SWDGE staging adds ~20us of trigger
                # latency); only the collective itself runs on gpsimd.
                nc.sync.dma_start(v_in[:], VL_sb[:, half * HC:(half + 1) * HC])
                nc.gpsimd.collective_compute(
                    "AllGather", mybir.AluOpType.bypass,
                    replica_groups=GROUPS,
                    ins=[v_in.opt()], outs=[v_out.opt()])
                nc.sync.dma_start(V_sb[:, half * HC:(half + 1) * HC], v_out[0])
                nc.sync.dma_start(V_sb[:, DC + half * HC:DC + (half + 1) * HC],
                                  v_out[1])

            # ---- scores -> softmax -> +(I^T-0.5) -> attn_c^T (fp8) ----
            # Transposes for si are emitted after the scores block of
            # si+1, so diag(si) is ready by the time the PE gets there.
            # BIAS tiles for the PV phase prefetch during this window.
            B_tiles = []
            for dvi in range(DC):
                B_sb = bias_pool.tile([P, SH], f32, tag="bias")
                nc.sync.dma_start(B_sb[:], bias_v[:, dvi, :])
                B_tiles.append(B_sb)
            if True:
                pend = None  # (si, E_sb, IT_sb, diag)

                def emit_transposes(si, E_sb, IT_sb, diag):
                    for t in range(TC):
                        pt = tr_ps.tile([P, P], f32, tag="tr")
                        nc.tensor.matmul(
                            pt[:], E_sb[:, t * P:(t + 1) * P], diag[:],
                            start=True, stop=True)
                        nc.vector.tensor_tensor(
                            AT_sb[:, t, si * P:(si + 1) * P],
                            pt[:], IT_sb[:, t * P:(t + 1) * P], add)

                for si in range(DC):
                    if si in IT_tiles:
                        IT_sb = IT_tiles[si]
                    else:
                        IT_sb = it_pool.tile([P, S], bf16, tag="it")
                        nc.sync.dma_start(IT_sb[:], it_v[si])
                    E_sb = e_pool.tile([P, S], bf16, tag="e")
                    ACC = stat_pool.tile([P, TJ], f32, tag="acc")
                    psl = [mm_ps.tile([P, NT], f32, tag="mm", name="ps")
                           for _ in range(TJ)]
                    for dc2 in range(DC2):
                        for tj in range(TJ):
                            nc.tensor.matmul(
                                psl[tj][:],
                                QT_sb[:, 2 * dc2:2 * dc2 + 2, si * P:(si + 1) * P],
                                KT_sb[:, tj // 2, dc2 // 2,
                                      2 * (dc2 % 2):2 * (dc2 % 2) + 2,
                                      (tj % 2) * NT:(tj % 2 + 1) * NT],
                                start=(dc2 == 0), stop=(dc2 == DC2 - 1),
                                perf_mode=DR,
                            )
                    for tj in range(TJ):
                        nc.scalar.activation(
                            E_sb[:, tj * NT:(tj + 1) * NT], psl[tj][:], Exp,
                            scale=SCALE, accum_out=ACC[:, tj:tj + 1],
                        )
                    den = stat_pool.tile([P, 1], f32, tag="den")
                    recip = stat_pool.tile([P, 1], f32, tag="recip")
                    diag = stat_pool.tile([P, P], bf16, tag="diag")
                    nc.vector.reduce_sum(
                        den[:], ACC[:], axis=mybir.AxisListType.X)
                    nc.vector.reciprocal(recip[:], den[:])
                    nc.vector.tensor_scalar_mul(diag[:], ident[:], recip[:])
                    if pend is not None:
                        emit_transposes(*pend)
                    pend = (si, E_sb, IT_sb, diag)
                emit_transposes(*pend)

            # ---- PV (fp8 DoubleRow): out^T = V_c.T @ attn_c^T + BIAS --
            for dvi in range(DC):
                B_sb = B_tiles[dvi]
                psl = [mm_ps.tile([P, NT], f32, tag="mm", name="ps")
                       for _ in range(SJ)]
                for tc2 in range(TC2):
                    for sj in range(SJ):
                        nc.tensor.matmul(
                            psl[sj][:],
                            V_sb[:, 2 * tc2:2 * tc2 + 2, dvi * P:(dvi + 1) * P],
                            AT_sb[:, 2 * tc2:2 * tc2 + 2, sj * NT:(sj + 1) * NT],
                            start=(tc2 == 0), stop=(tc2 == TC2 - 1),
                            perf_mode=DR,
                        )
                for sj in range(SJ):
                    F_sb = fin_pool.tile([P, NT], f32, tag="fin")
                    nc.vector.tensor_tensor(
                        F_sb[:], psl[sj][:],
                        B_sb[:, sj * NT:(sj + 1) * NT], add)
                    nc.sync.dma_start(
                        out_v[:, dvi, sj * NT:(sj + 1) * NT], F_sb[:])

    nc.compile()
    return nc


def _get_module():
    if "nc" not in _CACHE:
        _CACHE["nc"] = _build_module()
    return _CACHE["nc"]


def _make_in_maps(inputs):
    X = np.asarray(inputs["X"], dtype=np.float32)
    intensity = np.asarray(inputs["intensity"], dtype=np.float32)
    bf = ml_dtypes.bfloat16
    f8 = ml_dtypes.float8_e4m3
    Wq = np.asarray(inputs["Wq"], np.float32)
    Wk = np.asarray(inputs["Wk"], np.float32)
    Wv = np.asarray(inputs["Wv"], np.float32)
    Wo = np.asarray(inputs["Wo"], np.float32)
    Wvo = (Wo.astype(np.float64) @ Wv.astype(np.float64))  # fused V/O proj
    WQ8 = np.ascontiguousarray((WS * Wq).T).astype(f8)
    WK8 = np.ascontiguousarray((WS * Wk).T).astype(f8)
    WVO16 = np.ascontiguousarray(Wvo.T.astype(np.float32)).astype(bf)
    bq, bk, bv, bo = (np.asarray(inputs[k], np.float32).reshape(D)
                      for k in ("bq", "bk", "bv", "bo"))
    bvo = (Wo.astype(np.float64) @ bv.astype(np.float64))
    BCOL = np.concatenate(
        [(WS * b).reshape(DC, P).T for b in (bq, bk)], axis=1
    ).astype(np.float32)  # [128, 16]

    in_maps = []
    for c in range(8):
        b, h = c // 2, c % 2
        XT = X[b].T                                   # [D, S]
        XQ8 = np.ascontiguousarray(XT[:, h * SH:(h + 1) * SH]).astype(f8)
        XH16 = np.ascontiguousarray(XT[:, h * SH:(h + 1) * SH]).astype(bf)
        # column mean of V' = colsum(X) @ Wvo.T / S, quantized to the
        # same bf16 the device subtracts so bias and centering agree.
        m = (X[b].sum(axis=0, dtype=np.float64) @ Wvo.T) / S
        m16 = m.astype(np.float32).astype(bf)
        M16 = np.broadcast_to(m16, (P, D)).copy()
        Islc = intensity[b, h * SH:(h + 1) * SH, :]
        # [t, s] -> [si*128+tp, tc*128+sp] so each per-si load is one
        # contiguous row-block (128 descriptors instead of 2048)
        IT16 = np.ascontiguousarray(
            (Islc.T - 0.5).reshape(TC, P, DC, P)
            .transpose(2, 1, 0, 3).reshape(SH, S)
        ).astype(bf)
        rows = 1.0 + Islc.sum(axis=1, dtype=np.float64)
        BIAS = ((bvo + m16.astype(np.float64))[:, None] * rows[None, :]
                + bo.astype(np.float64)[:, None]).astype(np.float32)
        in_maps.append({
            "XQ8": XQ8, "XH16": XH16,
            "WQ8": WQ8, "WK8": WK8, "WVO16": WVO16, "M16": M16,
            "BCOL": BCOL, "BIAS": BIAS, "IT16": IT16,
        })
    return in_maps


def _gather(results):
    out = np.empty((4, S, D), dtype=np.float32)
    for c in range(8):
        b, h = c // 2, c % 2
        out[b, h * SH:(h + 1) * SH, :] = results[c]["OUTT"].T
    return out


def kernel(**inputs):
    from concourse import bass_utils

    in_maps = _make_in_maps(inputs)
    nc = _get_module()
    res = bass_utils.run_bass_kernel_spmd(nc, in_maps, core_ids=list(range(8)))
    return _gather(res.results)


# revision 29
# speedup vs baseline: 1.0646x; 1.0630x over previous
"""Trainium2 Bass kernel for nn_Attention_54254026883778.

Single-head attention with an additive post-softmax intensity term:
    q/k/v = X @ W{q,k,v}.T + b;  scores = q k^T / sqrt(D)
    attn  = softmax(scores) + intensity;  out = (attn @ v) @ Wo.T + bo

Sharding: 8 cores = 4 batches x 2 sequence halves. Each core computes
Q^T, K^T and V' for its own 1024 rows; the K and V halves are exchanged
within each batch pair through staged 2-rank AllGathers (two 0.5MB
gathers each, pipelined so they hide under the Q/V'/scores compute).

Math restructuring (host side, exact):
    Wvo = Wo @ Wv  =>  out = attn @ (X @ Wvo.T) + rowsum(attn) (x) (Wo@bv) + bo
which deletes the output projection GEMM. rowsum(attn) = 1 + rowsum(I)
is shipped from the host.

Precision: all large GEMMs run in fp8 e4m3 with MatmulPerfMode.DoubleRow
(256-row contraction per instruction, 2x bf16 throughput):
  - Q/K projections + scores: weights pre-scaled by 32 on the host so
    their uniform(+-1/32) entries stay in the e4m3 normal range; the
    32*32 factor is folded into the softmax exp scale. The softmax
    contribution to the output is tiny (intensity dominates attn).
  - PV runs fp8 on DOUBLE-CENTERED operands: attn_c = attn - 0.5 (the
    host ships intensity^T - 0.5) and V_c = V' - m, where m is the
    column mean of V' (host-exact: m = colsum(X) @ Wvo.T / S, quantized
    to the same bf16 the device subtracts). Centering routes the large
    attn/V' means through the exact f32 bias tensor instead of the fp8
    operands, which is what keeps the quantization noise acceptable:
       BIAS[do,s] = (Wo@bv + m)[do] * rowsums[s] + bo[do]
    (the m (x) rowsums term reconstructs both centering corrections).
  - V'-proj itself stays bf16 (X, Wvo bf16): V'-path input errors are
    amplified by ~sqrt(S)*|attn| in attn @ V', so fp8 inputs there
    would blow the error budget; fp8 only on the PV operand side.

Device dataflow (all t in global order; per-core tensors from host):
    K^T  [dout | t-own] = WK8-chunk.T @ XQ8    (fp8 DR, own rows)
        -> two staged AllGathers (dout-chunks 0..3, then 4..7) -> K full
    Q^T  [dout | s]   = WQ8-chunk.T @ XQ8      (fp8 DR, own rows)
    V'   [t-own | dv] = XH16-chunk.T @ WVO16   (bf16, own half) - m -> fp8
        -> two staged AllGathers (t-chunks 0..3, then 4..7) -> V full
    scores [s | t]    = QT-chunk.T @ KT        (fp8 DR) -> exp on ACT
        (scale 1/32768, no max-subtract: |scores|<~3) with fused
        row-accumulate -> 1/den on DVE -> diag(recip) = ident * recip ->
        attn^T tile = E-slice.T @ diag(recip)  (one PE matmul both
        transposes and normalizes) -> DVE drain adds (I^T - 0.5) -> fp8
        The transpose block for si runs after the scores block of si+1
        so the PE never waits on the exp->recip->diag chain.
    out^T [do | s]    = V-chunk.T @ attn_c^T   (fp8 DR) -> DVE adds the
        host BIAS tile while draining PSUM -> DRAM (f32), host transposes.
"""

import numpy as np
import ml_dtypes

P = 128
D = 1024
S = 2048          # keys per batch (full sequence)
SH = 1024         # query rows owned by each core
DC = D // P       # 8  contraction chunks over model dim
DC2 = DC // 2     # 4  DoubleRow chunk-pairs
TC = S // P       # 16 t (key) chunks
TC2 = TC // 2     # 8  DoubleRow t-chunk pairs
NT = 512          # matmul moving free dim / psum bank
SJ = SH // NT     # 2  s-tiles of own rows
TJ = S // NT      # 4  t-tiles
WS = 32.0         # host pre-scale on Wq/Wk (keeps fp8 in normal range)
SCALE = 1.0 / (32.0 * WS * WS)  # 1/sqrt(D) / WS^2

_CACHE = {}


def _build_module():
    import concourse.bass as bass
    import concourse.tile as tile
    import concourse.mybir as mybir
    from concourse import bacc
    from concourse.masks import make_identity

    f32 = mybir.dt.float32
    bf16 = mybir.dt.bfloat16
    fp8 = mybir.dt.float8e4
    DR = mybir.MatmulPerfMode.DoubleRow
    Exp = mybir.ActivationFunctionType.Exp
    add = mybir.AluOpType.add
    sub = mybir.AluOpType.subtract

    nc = bacc.Bacc("TRN2", target_bir_lowering=False, debug=False,
                   num_devices=8)

    XQ_d = nc.dram_tensor("XQ8", [D, SH], fp8, kind="ExternalInput")
    XH_d = nc.dram_tensor("XH16", [D, SH], bf16, kind="ExternalInput")
    WQ_d = nc.dram_tensor("WQ8", [D, D], fp8, kind="ExternalInput")
    WK_d = nc.dram_tensor("WK8", [D, D], fp8, kind="ExternalInput")
    WVO_d = nc.dram_tensor("WVO16", [D, D], bf16, kind="ExternalInput")
    M_d = nc.dram_tensor("M16", [P, D], bf16, kind="ExternalInput")
    BCOL_d = nc.dram_tensor("BCOL", [P, 2 * DC], f32, kind="ExternalInput")
    BIAS_d = nc.dram_tensor("BIAS", [D, SH], f32, kind="ExternalInput")
    IT_d = nc.dram_tensor("IT16", [SH, S], bf16, kind="ExternalInput")
    OUT_d = nc.dram_tensor("OUTT", [D, SH], f32, kind="ExternalOutput")

    xq_v = XQ_d[:].rearrange("(c p) s -> p c s", p=P)
    xh_v = XH_d[:].rearrange("(c p) s -> p c s", p=P)
    wq_v = WQ_d[:].rearrange("(c p) o -> p c o", p=P)
    wk_v = WK_d[:].rearrange("(c p) o -> p c o", p=P)
    wvo_v = WVO_d[:].rearrange("(c p) o -> p c o", p=P)
    it_v = IT_d[:].rearrange("(si p) f -> si p f", p=P)   # per-si blocks
    bias_v = BIAS_d[:].rearrange("(c p) s -> p c s", p=P)
    out_v = OUT_d[:].rearrange("(c p) s -> p c s", p=P)

    GROUPS = [[0, 1], [2, 3], [4, 5], [6, 7]]

    with tile.TileContext(nc) as tc:
        with (
            tc.tile_pool(name="persist", bufs=1) as persist,
            tc.tile_pool(name="mm_ps", bufs=5, space="PSUM") as mm_ps,
            tc.tile_pool(name="tr_ps", bufs=3, space="PSUM") as tr_ps,
            tc.tile_pool(name="dram", bufs=1, space="DRAM") as dram_pool,
            tc.tile_pool(name="e_pool", bufs=2) as e_pool,
            tc.tile_pool(name="it_pool", bufs=3) as it_pool,
            tc.tile_pool(name="stat_pool", bufs=2) as stat_pool,
            tc.tile_pool(name="bias_pool", bufs=8) as bias_pool,
            tc.tile_pool(name="fin_pool", bufs=3) as fin_pool,
        ):
            # ---- persistent tiles -------------------------------------
            WK_sb = persist.tile([P, DC, D], fp8)
            WQ_sb = persist.tile([P, DC, D], fp8)
            KL_sb = persist.tile([P, DC, SH], fp8)     # own K^T [dout | t]
            XQ_sb = persist.tile([P, DC, SH], fp8)
            WVO_sb = persist.tile([P, DC, D], bf16)
            XH_sb = persist.tile([P, DC, SH], bf16)
            # K^T [dout | rank-half, gather-half, dc-in-half, t]; the
            # extra split keeps each gather-output copy contiguous per
            # partition (4KB lines).
            KT_sb = persist.tile([P, 2, 2, DC // 2, SH], fp8)
            QT_sb = persist.tile([P, DC, SH], fp8)     # Q^T [dout | s]
            VL_sb = persist.tile([P, DC, SH], fp8)     # own V_c [t | dv]
            V_sb = persist.tile([P, TC, D], fp8)       # full V_c [t | dv]
            AT_sb = persist.tile([P, TC, SH], fp8)     # attn_c^T [t | s]
            M_sb = persist.tile([P, D], bf16)          # colmean(V') rows
            ident = persist.tile([P, P], bf16)
            BCOL_sb = persist.tile([P, 2 * DC], f32)

            # K operands first (the K exchange is the longest dependency
            # chain), then V', then Q. The first matmul's payload is
            # split extra-fine (WK pair0 col-block + XQ pair0 halves) so
            # the PE starts as soon as ~160KB has landed. Chunk-pair
            # loads keep per-partition lines at 2KB+ for DMA efficiency.
            nc.sync.dma_start(WK_sb[:, 0:2, 0:P], wk_v[:, 0:2, 0:P])
            for sj in range(SJ):
                nc.sync.dma_start(XQ_sb[:, 0:2, sj * NT:(sj + 1) * NT],
                                  xq_v[:, 0:2, sj * NT:(sj + 1) * NT])
            nc.sync.dma_start(WK_sb[:, 0:2, P:D], wk_v[:, 0:2, P:D])
            for dc2 in range(1, DC2):
                nc.sync.dma_start(WK_sb[:, 2 * dc2:2 * dc2 + 2, :],
                                  wk_v[:, 2 * dc2:2 * dc2 + 2, :])
                nc.sync.dma_start(XQ_sb[:, 2 * dc2:2 * dc2 + 2, :],
                                  xq_v[:, 2 * dc2:2 * dc2 + 2, :])
            make_identity(nc, ident)
            nc.sync.dma_start(BCOL_sb[:], BCOL_d[:])
            nc.sync.dma_start(M_sb[:], M_d[:])
            for dc2 in range(DC2):
                nc.sync.dma_start(WQ_sb[:, 2 * dc2:2 * dc2 + 2, :],
                                  wq_v[:, 2 * dc2:2 * dc2 + 2, :])
            for dc in range(DC):
                nc.sync.dma_start(WVO_sb[:, dc, :], wvo_v[:, dc, :])
                nc.sync.dma_start(XH_sb[:, dc, :], xh_v[:, dc, :])

            # ---- K^T (fp8 DoubleRow) own rows, then pair AllGather ----
            # The exchange goes out in two 0.5MB halves (dout-chunks 0-3
            # and 4-7) so the first gather can start as soon as the
            # runtime's init barrier clears the CC stream.
            HK = DC // 2
            for c in range(DC):
                psl = [mm_ps.tile([P, NT], f32, tag="mm", name="ps")
                       for _ in range(SJ)]
                for dc2 in range(DC2):
                    for sj in range(SJ):
                        nc.tensor.matmul(
                            psl[sj][:],
                            WK_sb[:, 2 * dc2:2 * dc2 + 2, c * P:(c + 1) * P],
                            XQ_sb[:, 2 * dc2:2 * dc2 + 2, sj * NT:(sj + 1) * NT],
                            start=(dc2 == 0), stop=(dc2 == DC2 - 1),
                            perf_mode=DR,
                        )
                for sj in range(SJ):
                    nc.vector.tensor_scalar_add(
                        KL_sb[:, c, sj * NT:(sj + 1) * NT], psl[sj][:],
                        BCOL_sb[:, DC + c:DC + c + 1])
                if c % HK == HK - 1:
                    kh = c // HK
                    k_in = dram_pool.tile([P, HK, SH], fp8, name=f"k_in{kh}")
                    k_out = dram_pool.tile([2, P, HK, SH], fp8,
                                           name=f"k_out{kh}")
                    nc.sync.dma_start(k_in[:], KL_sb[:, kh * HK:(kh + 1) * HK])
                    nc.gpsimd.collective_compute(
                        "AllGather", mybir.AluOpType.bypass,
                        replica_groups=GROUPS,
                        ins=[k_in.opt()], outs=[k_out.opt()])
                    # rank order == global t order: half 0 even, half 1 odd
                    nc.sync.dma_start(KT_sb[:, 0, kh], k_out[0])
                    nc.sync.dma_start(KT_sb[:, 1, kh], k_out[1])

            # ---- Q^T (fp8 DoubleRow), own rows ------------------------
            for c in range(DC):
                psl = [mm_ps.tile([P, NT], f32, tag="mm", name="ps")
                       for _ in range(SJ)]
                for dc2 in range(DC2):
                    for sj in range(SJ):
                        nc.tensor.matmul(
                            psl[sj][:],
                            WQ_sb[:, 2 * dc2:2 * dc2 + 2, c * P:(c + 1) * P],
                            XQ_sb[:, 2 * dc2:2 * dc2 + 2, sj * NT:(sj + 1) * NT],
                            start=(dc2 == 0), stop=(dc2 == DC2 - 1),
                            perf_mode=DR,
                        )
                for sj in range(SJ):
                    nc.vector.tensor_scalar_add(
                        QT_sb[:, c, sj * NT:(sj + 1) * NT], psl[sj][:],
                        BCOL_sb[:, c:c + 1])

            # Prefetch the first IT tiles now so their DMAs sit ahead
            # of the V-exchange staging/copies in the sync queues.
            IT_tiles = {}
            for si in range(3):
                IT_tiles[si] = it_pool.tile([P, S], bf16, tag="it",
                                            name=f"it{si}")
                nc.sync.dma_start(IT_tiles[si][:], it_v[si])

            # ---- V' (bf16) own half, centered -> fp8; staged exchange -
            for half in range(2):
                for t in range(half * DC // 2, (half + 1) * DC // 2):
                    psl = [mm_ps.tile([P, NT], f32, tag="mm", name="ps")
                           for _ in range(D // NT)]
                    for dc in range(DC):
                        for j in range(D // NT):
                            nc.tensor.matmul(
                                psl[j][:],
                                XH_sb[:, dc, t * P:(t + 1) * P],
                                WVO_sb[:, dc, j * NT:(j + 1) * NT],
                                start=(dc == 0), stop=(dc == DC - 1),
                            )
                    for j in range(D // NT):
                        nc.vector.tensor_tensor(
                            VL_sb[:, t, j * NT:(j + 1) * NT],
                            psl[j][:], M_sb[:, j * NT:(j + 1) * NT], sub)
                HC = DC // 2
                v_in = dram_pool.tile([P, HC, SH], fp8, name=f"v_in{half}")
                v_out = dram_pool.tile([2, P, HC, SH], fp8,
                                       name=f"v_out{half}")
                # Stage via HWDGE (SWDGE staging adds ~20us of trigger
                # latency); only the collective itself runs on gpsimd.
                nc.sync.dma_start(v_in[:], VL_sb[:, half * HC:(half + 1) * HC])
                nc.gpsimd.collective_compute(
                    "AllGather", mybir.AluOpType.bypass,
                    replica_groups=GROUPS,
                    ins=[v_in.opt()], outs=[v_out.opt()])
                nc.sync.dma_start(V_sb[:, half * HC:(half + 1) * HC], v_out[0])
                nc.sync.dma_start(V_sb[:, DC + half * HC:DC + (half + 1) * HC],
                                  v_out[1])

            # ---- scores -> softmax -> +(I^T-0.5) -> attn_c^T (fp8) ----
            # Transposes for si are emitted after the scores block of
            # si+1, so diag(si) is ready by the time the PE gets there.
            # BIAS tiles for the PV phase prefetch during this window.
            B_tiles = []
            for dvi in range(DC):
                B_sb = bias_pool.tile([P, SH], f32, tag="bias")
                nc.sync.dma_start(B_sb[:], bias_v[:, dvi, :])
                B_tiles.append(B_sb)
            if True:
                pend = None  # (si, E_sb, IT_sb, diag)

                def emit_transposes(si, E_sb, IT_sb, diag):
                    for t in range(TC):
                        pt = tr_ps.tile([P, P], f32, tag="tr")
                        nc.tensor.matmul(
                            pt[:], E_sb[:, t * P:(t + 1) * P], diag[:],
                            start=True, stop=True)
                        nc.vector.tensor_tensor(
                            AT_sb[:, t, si * P:(si + 1) * P],
                            pt[:], IT_sb[:, t * P:(t + 1) * P], add)

                for si in range(DC):
                    if si in IT_tiles:
                        IT_sb = IT_tiles[si]
                    else:
                        IT_sb = it_pool.tile([P, S], bf16, tag="it")
                        nc.sync.dma_start(IT_sb[:], it_v[si])
                    E_sb = e_pool.tile([P, S], bf16, tag="e")
                    ACC = stat_pool.tile([P, TJ], f32, tag="acc")
                    psl = [mm_ps.tile([P, NT], f32, tag="mm", name="ps")
                           for _ in range(TJ)]
                    for dc2 in range(DC2):
                        for tj in range(TJ):
                            nc.tensor.matmul(
                                psl[tj][:],
                                QT_sb[:, 2 * dc2:2 * dc2 + 2, si * P:(si + 1) * P],
                                KT_sb[:, tj // 2, dc2 // 2,
                                      2 * (dc2 % 2):2 * (dc2 % 2) + 2,
                                      (tj % 2) * NT:(tj % 2 + 1) * NT],
                                start=(dc2 == 0), stop=(dc2 == DC2 - 1),
                                perf_mode=DR,
                            )
                    for tj in range(TJ):
                        nc.scalar.activation(
                            E_sb[:, tj * NT:(tj + 1) * NT], psl[tj][:], Exp,
                            scale=SCALE, accum_out=ACC[:, tj:tj + 1],
                        )
                    den = stat_pool.tile([P, 1], f32, tag="den")
                    recip = stat_pool.tile([P, 1], f32, tag="recip")
                    diag = stat_pool.tile([P, P], bf16, tag="diag")
                    nc.vector.reduce_sum(
                        den[:], ACC[:], axis=mybir.AxisListType.X)
                    nc.vector.reciprocal(recip[:], den[:])
                    nc.vector.tensor_scalar_mul(diag[:], ident[:], recip[:])
                    if pend is not None:
                        emit_transposes(*pend)
                    pend = (si, E_sb, IT_sb, diag)
                emit_transposes(*pend)

            # ---- PV (fp8 DoubleRow): out^T = V_c.T @ attn_c^T + BIAS --
            for dvi in range(DC):
                B_sb = B_tiles[dvi]
                psl = [mm_ps.tile([P, NT], f32, tag="mm", name="ps")
                       for _ in range(SJ)]
                for tc2 in range(TC2):
                    for sj in range(SJ):
                        nc.tensor.matmul(
                            psl[sj][:],
                            V_sb[:, 2 * tc2:2 * tc2 + 2, dvi * P:(dvi + 1) * P],
                            AT_sb[:, 2 * tc2:2 * tc2 + 2, sj * NT:(sj + 1) * NT],
                            start=(tc2 == 0), stop=(tc2 == TC2 - 1),
                            perf_mode=DR,
                        )
                for sj in range(SJ):
                    F_sb = fin_pool.tile([P, NT], f32, tag="fin")
                    nc.vector.tensor_tensor(
                        F_sb[:], psl[sj][:],
                        B_sb[:, sj * NT:(sj + 1) * NT], add)
                    nc.sync.dma_start(
                        out_v[:, dvi, sj * NT:(sj + 1) * NT], F_sb[:])

    nc.compile()
    return nc


def _get_module():
    if "nc" not in _CACHE:
        _CACHE["nc"] = _build_module()
    return _CACHE["nc"]


def _make_in_maps(inputs):
    X = np.asarray(inputs["X"], dtype=np.float32)
    intensity = np.asarray(inputs["intensity"], dtype=np.float32)
    bf = ml_dtypes.bfloat16
    f8 = ml_dtypes.float8_e4m3
    Wq = np.asarray(inputs["Wq"], np.float32)
    Wk = np.asarray(inputs["Wk"], np.float32)
    Wv = np.asarray(inputs["Wv"], np.float32)
    Wo = np.asarray(inputs["Wo"], np.float32)
    Wvo = (Wo.astype(np.float64) @ Wv.astype(np.float64))  # fused V/O proj
    WQ8 = np.ascontiguousarray((WS * Wq).T).astype(f8)
    WK8 = np.ascontiguousarray((WS * Wk).T).astype(f8)
    WVO16 = np.ascontiguousarray(Wvo.T.astype(np.float32)).astype(bf)
    bq, bk, bv, bo = (np.asarray(inputs[k], np.float32).reshape(D)
                      for k in ("bq", "bk", "bv", "bo"))
    bvo = (Wo.astype(np.float64) @ bv.astype(np.float64))
    BCOL = np.concatenate(
        [(WS * b).reshape(DC, P).T for b in (bq, bk)], axis=1
    ).astype(np.float32)  # [128, 16]

    in_maps = []
    for c in range(8):
        b, h = c // 2, c % 2
        XT = X[b].T                                   # [D, S]
        XQ8 = np.ascontiguousarray(XT[:, h * SH:(h + 1) * SH]).astype(f8)
        XH16 = np.ascontiguousarray(XT[:, h * SH:(h + 1) * SH]).astype(bf)
        # column mean of V' = colsum(X) @ Wvo.T / S, quantized to the
        # same bf16 the device subtracts so bias and centering agree.
        m = (X[b].sum(axis=0, dtype=np.float64) @ Wvo.T) / S
        m16 = m.astype(np.float32).astype(bf)
        M16 = np.broadcast_to(m16, (P, D)).copy()
        Islc = intensity[b, h * SH:(h + 1) * SH, :]
        # [t, s] -> [si*128+tp, tc*128+sp] so each per-si load is one
        # contiguous row-block (128 descriptors instead of 2048)
        IT16 = np.ascontiguousarray(
            (Islc.T - 0.5).reshape(TC, P, DC, P)
            .transpose(2, 1, 0, 3).reshape(SH, S)
        ).astype(bf)
        rows = 1.0 + Islc.sum(axis=1, dtype=np.float64)
        BIAS = ((bvo + m16.astype(np.float64))[:, None] * rows[None, :]
                + bo.astype(np.float64)[:, None]).astype(np.float32)
        in_maps.append({
            "XQ8": XQ8, "XH16": XH16,
            "WQ8": WQ8, "WK8": WK8, "WVO16": WVO16, "M16": M16,
            "BCOL": BCOL, "BIAS": BIAS, "IT16": IT16,
        })
    return in_maps


def _gather(results):
    out = np.empty((4, S, D), dtype=np.float32)
    for c in range(8):
        b, h = c // 2, c % 2
        out[b, h * SH:(h + 1) * SH, :] = results[c]["OUTT"].T
    return out


def kernel(**inputs):
    from concourse import bass_utils

    in_maps = _make_in_maps(inputs)
    nc = _get_module()
    res = bass_utils.run_bass_kernel_spmd(nc, in_maps, core_ids=list(range(8)))
    return _gather(res.results)
